# revision 29
# baseline (speedup 1.0000x reference)
"""Multi-head attention TRN2 Bass kernel.

Problem: B=8, S=1024, D=768, H=12 heads of DH=64 (torch-style per-head
Linear Q/K/V, softmax over keys, attn @ V, heads concatenated).

Sharding: data-parallel over batch - one batch element per NeuronCore
(8 cores). Each core computes its full [1024, 768] output slice; the host
gathers by stacking.

Per-core kernel strategy:
  - Host pre-transposes x to xT [768, 1024] and builds block-diagonal
    head-PAIR weights so all projection matmuls run with K=128.
  - Q/K are produced transposed (QT/KT [d, s]) which is what the scores
    matmul wants; V is produced in natural [t, d] layout (with an all-ones
    column wedged between the two heads of a pair: [V_h0 | 1 | V_h1]).
  - Scores are computed transposed, scoresT [t, s] = KT.T @ QT, two heads
    of a pair concurrently in the two 64-row halves of the PE array.
  - exp() runs on the scalar engine straight out of PSUM (scale=1/sqrt(64)
    folded into the activation's free affine). No max-subtraction: scores
    for these inputs are bounded (|s| < ~10), exp is safe in fp32, and
    softmax is shift-invariant so the result matches the reference.
  - AV: out_T[d, s] (+ denominator row, from the ones column) accumulates
    over t-chunks in PSUM with exp tiles as the moving operand.
  - Final [65, 128] chunks are transposed back on the tensor engine,
    normalized by 1/denominator (vector engine, per-partition scalar) into
    a [128, 768] staging tile, biased (bv) and DMA'd out.
"""

import numpy as np
import ml_dtypes

import concourse.bass as bass
import concourse.mybir as mybir
import concourse.tile as tile
from concourse import bacc
from concourse import bass_utils
from concourse.masks import make_identity

H, DH = 12, 64
B, S, D = 8, 1024, 768
NPAIR = H // 2          # head pairs (block-diagonal packing)
NCORES = 8
SHW = 512               # s-half width per attention sweep
NT = S // 128           # t-chunks per head (8)
VW = 130                # V sbuf stride per t-chunk: [V_h0(64) | 1 | V_h1(64) | pad]

F32 = mybir.dt.float32
DT = mybir.dt.bfloat16  # matmul operand dtype
NPDT = ml_dtypes.bfloat16
AF = mybir.ActivationFunctionType


def _emit(ctx, tc, nc, xT, wqk, wv, bqk, bvf, out, reps=1, dummy=None):
    P = 128
    const = ctx.enter_context(tc.tile_pool(name="const", bufs=1))
    xpool = ctx.enter_context(tc.tile_pool(name="xpool", bufs=1))
    qkpool = ctx.enter_context(tc.tile_pool(name="qkpool", bufs=1))
    vpool = ctx.enter_context(tc.tile_pool(name="vpool", bufs=1))
    opool = ctx.enter_context(tc.tile_pool(name="opool", bufs=1))
    expp = ctx.enter_context(tc.tile_pool(name="expp", bufs=16))
    otp = ctx.enter_context(tc.tile_pool(name="otp", bufs=3))
    rcp = ctx.enter_context(tc.tile_pool(name="rcp", bufs=3))
    psum = ctx.enter_context(tc.tile_pool(name="psum", bufs=1, space="PSUM"))

    if dummy is not None:
        dtile = const.tile([1, dummy.shape[1]], F32, tag="dummy")
        nc.sync.dma_start(out=dtile[:], in_=dummy[:])
    # ---- constants (DMA order: needed-first) ----
    wqk_t = const.tile([P, 2 * NPAIR * P], DT, tag="wqk")
    nc.sync.dma_start(out=wqk_t[:], in_=wqk[:])
    bias_t = const.tile([P, 2 * NPAIR], F32, tag="bqk")
    nc.sync.dma_start(out=bias_t[:], in_=bqk[:])
    wv_t = const.tile([P, NPAIR * 129], DT, tag="wv")
    nc.sync.dma_start(out=wv_t[:], in_=wv[:])
    ident = const.tile([P, P], F32, tag="ident")
    make_identity(nc, ident)
    bvf_t = const.tile([P, D], F32, tag="bvf")
    nc.sync.dma_start(out=bvf_t[:], in_=bvf[:])

    # ---- x tiles ----
    xt = []

    def emit_x():
        xt.clear()
        for p in range(NPAIR):
            t = xpool.tile([P, S], DT, tag=f"x{p}", name=f"x{p}")
            nc.sync.dma_start(out=t[:], in_=xT[P * p : P * (p + 1), :])
            xt.append(t)

    # ---- output staging ----
    out_sb = [
        opool.tile([P, D], F32, tag=f"o{j}", name=f"o{j}") for j in range(S // P)
    ]

    # ---- projections (emitted per-pair, interleaved with attention) ----
    QT, KT, VS = [], [], []

    def emit_qk(p):
        qt = qkpool.tile([P, S], DT, tag=f"q{p}", name=f"q{p}")
        kt = qkpool.tile([P, S], DT, tag=f"k{p}", name=f"k{p}")
        for which, dst, sh in ((0, qt, 0), (1, kt, 0), (1, kt, 1), (0, qt, 1)):
            wcol = 2 * p + which
            if True:
                ps = psum.tile([P, SHW], F32, tag="avt", bufs=4, name="pjqk")
                nc.tensor.matmul(
                    ps[:],
                    wqk_t[:, wcol * P : (wcol + 1) * P],
                    xt[p][:, SHW * sh : SHW * (sh + 1)],
                    start=True,
                    stop=True,
                )
                nc.vector.tensor_scalar_add(
                    dst[:, SHW * sh : SHW * (sh + 1)],
                    ps[:],
                    bias_t[:, wcol : wcol + 1],
                )
        QT.append(qt)
        KT.append(kt)

    def emit_v(p):
        vs = vpool.tile([P, NT * VW], DT, tag=f"v{p}", name=f"v{p}")
        # ones column between the two heads' V blocks, once per t-chunk
        nc.vector.memset(
            vs[:].rearrange("p (a b) -> p a b", a=NT, b=VW)[:, :, 64:65], 1.0
        )
        for c in range(NT):
            pv = psum.tile([P, VW], F32, tag="avt", bufs=4, name="pjv")
            nc.tensor.matmul(
                pv[:, 0:129],
                xt[p][:, P * c : P * (c + 1)],
                wv_t[:, p * 129 : (p + 1) * 129],
                start=True,
                stop=True,
            )
            dst = vs[:, VW * c : VW * (c + 1)].rearrange(
                "p (a b) -> p a b", a=2, b=65
            )[:, :, 0:64]
            src = pv[:].rearrange("p (a b) -> p a b", a=2, b=65)[:, :, 0:64]
            nc.vector.tensor_copy(dst, src)
        VS.append(vs)

    def attn_scores(p, sh):
        """scores (transposed) + exp, in [128, 1024] two-t-chunk groups."""
        qt, kt = QT[p], KT[p]
        exps = {}
        for g in range(NT // 2):
            pg = [
                psum.tile([P, 1024], F32, tag="sc", bufs=2, name=f"sc{h2}")
                for h2 in range(2)
            ]
            for h2 in range(2):
                for tt in range(2):
                    tau = 2 * g + tt
                    nc.tensor.matmul(
                        pg[h2][:, 512 * tt : 512 * (tt + 1)],
                        kt[64 * h2 : 64 * (h2 + 1), P * tau : P * (tau + 1)],
                        qt[64 * h2 : 64 * (h2 + 1), SHW * sh : SHW * (sh + 1)],
                        start=True,
                        stop=True,
                    )
                et = expp.tile([P, 1024], DT, tag="exp", name="exp")
                nc.scalar.activation(et[:], pg[h2][:], AF.Exp, scale=0.125)
                exps[(h2, g)] = et
        return exps

    def attn_post(p, sh, exps):
        """AV + denominator row, transpose back, normalize into out_sb."""
        vs = VS[p]
        for h2 in range(2):
            pav = psum.tile([P, SHW], F32, tag="avt", bufs=4, name="pav")
            voff = 64 * h2  # h0: [V|1] at 0:65; h1: [1|V] at 64:129
            for tau in range(NT):
                et = exps[(h2, tau // 2)]
                nc.tensor.matmul(
                    pav[0:65, :],
                    vs[:, VW * tau + voff : VW * tau + voff + 65],
                    et[:, 512 * (tau % 2) : 512 * (tau % 2 + 1)],
                    start=(tau == 0),
                    stop=(tau == NT - 1),
                )
            ot = otp.tile([65, SHW], F32, tag="ot", name="ot")
            nc.vector.tensor_copy(ot[:], pav[0:65, :])
            pt = psum.tile([P, 4 * 65], F32, tag="avt", bufs=4, name="pt")
            for j in range(4):
                nc.tensor.transpose(
                    pt[:, 65 * j : 65 * (j + 1)],
                    ot[:, P * j : P * (j + 1)],
                    ident[0:65, 0:65],
                )
            dcol = 64 if h2 == 0 else 0  # denominator col within 65-block
            doff = 0 if h2 == 0 else 1  # data col offset within 65-block
            rc = rcp.tile([P, 4], F32, tag="rc", name="rc")
            nc.vector.reciprocal(
                rc[:],
                pt[:].rearrange("p (a b) -> p a b", a=4, b=65)[:, :, dcol],
            )
            hcol = 64 * (2 * p + h2)
            for j in range(4):
                nc.vector.tensor_scalar_mul(
                    out_sb[4 * sh + j][:, hcol : hcol + 64],
                    pt[:, 65 * j + doff : 65 * j + doff + 64],
                    rc[:, j : j + 1],
                )

    def emit_writeback(sh):
        for j in range(4):
            stile = 4 * sh + j
            nc.vector.tensor_add(out_sb[stile][:], out_sb[stile][:], bvf_t[:])
            nc.sync.dma_start(
                out=out[P * stile : P * (stile + 1), :], in_=out_sb[stile][:]
            )

    # ---- software-pipelined attention ----
    # scores/exp of iteration i+1 are emitted (and thus prioritized) before
    # AV/post of iteration i, so the scalar engine never starves between
    # pairs. Projections stream in two pairs ahead of the attention sweep.
    # reps>1 replicates the whole computation (timing-measurement builds).
    for _ in range(reps):
        QT.clear()
        KT.clear()
        VS.clear()
        emit_x()
        emit_qk(0)
        emit_qk(1)
        items = [(sh, p) for sh in range(2) for p in range(NPAIR)]
        pending = None
        for i, (sh, p) in enumerate(items):
            exps = attn_scores(p, sh)
            if sh == 0:
                emit_v(p)
            if i + 2 < len(items) and items[i + 2][0] == 0:
                emit_qk(items[i + 2][1])
            if pending is not None:
                attn_post(*pending)
                if pending[1] == 0 and pending[0] == NPAIR - 1:
                    emit_writeback(0)
            pending = (p, sh, exps)
        attn_post(*pending)
        emit_writeback(1)


_NC_CACHE = {}


def build_nc(reps=1):
    if reps in _NC_CACHE:
        return _NC_CACHE[reps]
    nc = bacc.Bacc("TRN2", target_bir_lowering=False, debug=False)
    if reps > 1:
        # distinct HLO signature so executable caches can't alias variants
        dummy = nc.dram_tensor("abreps", [1, 16 * reps], F32, kind="ExternalInput")
    xT = nc.dram_tensor("xT", [D, S], DT, kind="ExternalInput")
    wqk = nc.dram_tensor("wqk", [128, 2 * NPAIR * 128], DT, kind="ExternalInput")
    wv = nc.dram_tensor("wv", [128, NPAIR * 129], DT, kind="ExternalInput")
    bqk = nc.dram_tensor("bqk", [128, 2 * NPAIR], F32, kind="ExternalInput")
    bvf = nc.dram_tensor("bvf", [128, D], F32, kind="ExternalInput")
    out = nc.dram_tensor("out", [S, D], F32, kind="ExternalOutput")
    from contextlib import ExitStack

    with tile.TileContext(nc) as tc:
        with ExitStack() as ctx:
            _emit(
                ctx,
                tc,
                nc,
                xT[:],
                wqk,
                wv,
                bqk,
                bvf,
                out[:],
                reps=reps,
                dummy=dummy if reps > 1 else None,
            )
    nc.finalize()
    _NC_CACHE[reps] = nc
    return nc


def host_prep(sequences, Wq, bq, Wk, bk, Wv, bv):
    """Build the per-core input maps (host-side sharding + layout prep)."""
    sequences = np.asarray(sequences, np.float32)
    Wq, Wk, Wv = (np.asarray(a, np.float32) for a in (Wq, Wk, Wv))
    bq, bk, bv = (np.asarray(a, np.float32) for a in (bq, bk, bv))

    wqk = np.zeros((2 * NPAIR, 128, 128), np.float32)
    for p in range(NPAIR):
        for which, W in ((0, Wq), (1, Wk)):
            wqk[2 * p + which, 0:64, 0:64] = W[2 * p].T
            wqk[2 * p + which, 64:128, 64:128] = W[2 * p + 1].T
    # SBUF-final layout: [128 partitions, m*free]
    wqk = np.ascontiguousarray(wqk.transpose(1, 0, 2)).reshape(128, 2 * NPAIR * 128)
    wv_bd = np.zeros((NPAIR, 128, 129), np.float32)
    for p in range(NPAIR):
        wv_bd[p, 0:64, 0:64] = Wv[2 * p].T
        wv_bd[p, 64:128, 65:129] = Wv[2 * p + 1].T
    wv_bd = np.ascontiguousarray(wv_bd.transpose(1, 0, 2)).reshape(128, NPAIR * 129)
    bqk_t = np.zeros((128, 2 * NPAIR), np.float32)
    for p in range(NPAIR):
        bqk_t[0:64, 2 * p] = bq[2 * p]
        bqk_t[64:128, 2 * p] = bq[2 * p + 1]
        bqk_t[0:64, 2 * p + 1] = bk[2 * p]
        bqk_t[64:128, 2 * p + 1] = bk[2 * p + 1]
    bvf = np.tile(bv.reshape(1, D), (128, 1)).astype(np.float32)

    shared = {
        "wqk": wqk.astype(NPDT),
        "wv": wv_bd.astype(NPDT),
        "bqk": bqk_t,
        "bvf": bvf,
    }
    in_maps = []
    for b in range(NCORES):
        xTb = np.ascontiguousarray(sequences[b].T).astype(NPDT)
        in_maps.append({"xT": xTb, **shared})
    return in_maps


def kernel(**inputs):
    nc = build_nc()
    in_maps = host_prep(
        inputs["sequences"],
        inputs["Wq"],
        inputs["bq"],
        inputs["Wk"],
        inputs["bk"],
        inputs["Wv"],
        inputs["bv"],
    )
    res = bass_utils.run_bass_kernel_spmd(
        nc, in_maps, core_ids=list(range(NCORES))
    )
    return np.stack([r["out"] for r in res.results], axis=0).astype(np.float32)
